# revision 33
# baseline (speedup 1.0000x reference)
"""Causal self-attention with RoPE on 8 TRN2 NeuronCores — v2.

Head-parallel TP as v1 (core i owns heads 2i, 2i+1), redesigned around
the measured v1 bottlenecks:

- One PE stream with qkv/rope/V-transpose/outproj work software-pipelined
  INTO the attention kb-loop as filler, so PE never idles on the Act
  (exp) cadence and the Act engine never waits on phase boundaries.
- All PSUM [128,512] f32 tiles (qkv pq, rope pr, V-transpose, S, outproj
  po) share one 3-buffer pool (psS); avp accumulators keep 4 banks (psV);
  normalize-broadcast keeps 1 (psA).
- cos/sin loaded as TWO whole-tensor DMAs up-front (v1 chunk-loads built
  a 50us credit-semaphore chain that delayed the collectives); xt loads
  merged per (batch, c-block); staging is ONE DMA per q-chunk and
  att_load ONE DMA per (batch, head) via AP rearrange.
- V transposed on PE in bf16 (1 cyc/row), ones columns pre-memset into
  persistent v tiles so AV keeps the fused [65 x 512] denominator-row
  trick.
- PSUM->SBUF casts moved off the Act engine (exp is its critical work)
  onto DVE; big loads issue on the Act HW-DGE queue before exp starts,
  staging/att_load/out on the SP queue.
- Collectives issue on Pool, whose stream contains nothing else.
"""

import numpy as np
import ml_dtypes

import concourse.bass as bass
import concourse.mybir as mybir
import concourse.tile as tile
from concourse import bacc
from concourse.bass_utils import run_bass_kernel_spmd
from concourse.dve_ops import (RECIP_APPROX_FAST_CONSTS,
                               RECIPROCAL_APPROX_FAST)

F32 = mybir.dt.float32
BF16 = mybir.dt.bfloat16

B, T, C = 2, 2048, 1024
H, HD = 16, 64
NC = 8
HL = H // NC          # heads per core = 2
BT = B * T            # 4096
FQKV = 3 * HL * HD    # 384 rows of w_attn per core
TSH = BT // NC        # 512 output rows per core (256 per batch)
NCH = BT // 512       # 8 chunks of 512 t
ROPE_BASE = 10000.0
BF = ml_dtypes.bfloat16


def build():
    nc = bacc.Bacc(None, target_bir_lowering=False)

    xT_d = nc.dram_tensor("xT", [C, BT], BF16, kind="ExternalInput")
    wq_d = nc.dram_tensor("wqkvT", [C, FQKV], BF16, kind="ExternalInput")
    wp_d = nc.dram_tensor("wpT", [C, C], BF16, kind="ExternalInput")
    cos_d = nc.dram_tensor("cosT", [128, BT], BF16, kind="ExternalInput")
    sin_d = nc.dram_tensor("sinT", [128, BT], BF16, kind="ExternalInput")
    perm_d = nc.dram_tensor("permT", [128, 128], BF16, kind="ExternalInput")
    mask_d = nc.dram_tensor("masks", [4, 128, 512], BF16, kind="ExternalInput")
    id_d = nc.dram_tensor("identB", [128, 128], BF16, kind="ExternalInput")
    out_d = nc.dram_tensor("out", [TSH, C], BF16, kind="ExternalOutput")

    # block j of a2a_in = my 128 attention channels for core j's 256 t-rows
    # of batch b; block j of a2a_out = core j's channels for MY 256 t-rows.
    a2a_in = {(b_, h_): nc.dram_tensor(f"a2ain{b_}{h_}", [8, 64, 256], BF16)
              for b_ in range(2) for h_ in range(2)}
    a2a_out = {(b_, h_): nc.dram_tensor(f"a2aout{b_}{h_}", [8, 64, 256], BF16)
               for b_ in range(2) for h_ in range(2)}

    with tile.TileContext(nc) as tc:
        with (
            tc.tile_pool(name="persist", bufs=1) as pp,
            tc.tile_pool(name="work", bufs=4) as wk,
            tc.tile_pool(name="pts", bufs=20) as ptp,
            tc.tile_pool(name="psS", bufs=5, space="PSUM") as psS,
            tc.tile_pool(name="psV", bufs=2, space="PSUM") as psV,
            tc.tile_pool(name="psA", bufs=1, space="PSUM") as psA,
        ):
            # ---- constants / weights: few big DMAs, DRAM side rearranged
            # so all 8 c-blocks land in one SBUF tile per tensor ----
            wq_big = pp.tile([128, 8 * FQKV], BF16, name="wqb", tag="wqb")
            nc.sync.dma_start(
                wq_big[:], wq_d[:].rearrange("(c p) f -> p c f", p=128))
            wq_sb = [wq_big[:, c * FQKV:(c + 1) * FQKV] for c in range(8)]
            id_sb = pp.tile([128, 128], BF16, name="id_sb", tag="id_sb")
            nc.sync.dma_start(id_sb[:], id_d[:])
            perm_sb = pp.tile([128, 128], BF16, name="perm_sb", tag="perm_sb")
            nc.sync.dma_start(perm_sb[:], perm_d[:])
            # xt: [128, 8*2048], one generation per batch; halves split
            # across the two HW-DGE queues so batch 0 lands in ~6us
            xt_big = [None]

            def xt_load(b):
                # 4 DMAs, 2 per HW queue, so the first qkv accumulation
                # chain can chase the c-blocks as they land
                t = pp.tile([128, 8 * 2048], BF16, name=f"xt{b}", tag="xt")
                for q_ in range(4):
                    eng = nc.sync if q_ % 2 == 0 else nc.scalar
                    eng.dma_start(
                        t[:, q_ * 2 * 2048:(q_ + 1) * 2 * 2048],
                        xT_d[q_ * 256:(q_ + 1) * 256,
                             b * 2048:(b + 1) * 2048].rearrange(
                            "(c p) t -> p c t", p=128))
                xt_big[0] = t

            xt_load(0)
            cos_sb = pp.tile([128, BT], BF16, name="cos_sb", tag="cos_sb")
            nc.scalar.dma_start(cos_sb[:], cos_d[:])
            sin_sb = pp.tile([128, BT], BF16, name="sin_sb", tag="sin_sb")
            nc.scalar.dma_start(sin_sb[:], sin_d[:])
            mask_big = pp.tile([128, 4 * 512], BF16, name="maskb", tag="maskb")
            nc.scalar.dma_start(
                mask_big[:], mask_d[:].rearrange("m p c -> p m c"))

            ones_f = pp.tile([128, 1], F32, name="ones_f", tag="ones_f")
            nc.vector.memset(ones_f[:], 1.0)
            ones_r = pp.tile([1, HD], mybir.dt.float32r, name="ones_r",
                             tag="ones_r")
            nc.vector.tensor_copy(ones_r[:],
                                  ones_f[0:1, 0:1].broadcast_to((1, HD)))

            # persistent V tiles [t,d]: cols 64 and 129 stay the memset 1.0
            # (denominator rows of the fused AV); memsets on the idle Pool
            v_sb = []
            for kb in range(32):
                v = pp.tile([128, 2 * (HD + 1)], BF16, name=f"v{kb}",
                            tag=f"v{kb}")
                nc.gpsimd.memset(v[:], 1.0)
                v_sb.append(v)

            qtc = [pp.tile([128, 512], BF16, name=f"qtc{i}", tag=f"qtc{i}")
                   for i in range(NCH)]
            ktc = [pp.tile([128, 512], BF16, name=f"ktc{i}", tag=f"ktc{i}")
                   for i in range(NCH)]
            vtc = [pp.tile([128, 512], BF16, name=f"vtc{i}", tag=f"vtc{i}")
                   for i in range(NCH)]
            fdst = [qtc, ktc, vtc]

            att_sb = {}
            for b in range(2):
                att_sb[b] = pp.tile([128, 2048], BF16, name=f"att{b}",
                                    tag=f"att{b}")
            wp_big = [None]

            def wp_load():
                t = pp.tile([128, 8 * C], BF16, name="wpb", tag="wpb")
                nc.scalar.dma_start(
                    t[:], wp_d[:].rearrange("(c p) o -> p c o", p=128))
                wp_big[0] = t

            # ---- building blocks ----
            def qkv_mms(pq, b, f, tq, cs):
                xt = xt_big[0]
                for c in cs:
                    nc.tensor.matmul(
                        pq[:],
                        wq_big[:, c * FQKV + f * 128:c * FQKV + f * 128 + 128],
                        xt[:, c * 2048 + tq * 512:c * 2048 + tq * 512 + 512],
                        start=(c == 0), stop=(c == 7))

            def qkv_piece(b, f, tq):
                """one [128, 512] slice of the qkv projection for batch b;
                lead-in (b=0) casts go to the then-idle Act engine"""
                pq = psS.tile([128, 512], F32, name=f"pq{b}{f}{tq}",
                              tag="ps_s")
                qkv_mms(pq, b, f, tq, range(8))
                if b == 0:
                    nc.scalar.copy(fdst[f][b * 4 + tq][:], pq[:])
                else:
                    nc.vector.tensor_copy(fdst[f][b * 4 + tq][:], pq[:])

            def qkv_quanta(b, f, tq):
                """same piece as 4 filler quanta of ~2 matmuls (~430ns),
                sized to hide in the exp-latency bubble of one kb block"""
                hold = {}

                def q_first():
                    hold['pq'] = psS.tile([128, 512], F32,
                                          name=f"pq{b}{f}{tq}", tag="ps_s")
                    qkv_mms(hold['pq'], b, f, tq, (0, 1))

                def q_mid(cs):
                    def g():
                        qkv_mms(hold['pq'], b, f, tq, cs)
                    return g

                def q_last():
                    qkv_mms(hold['pq'], b, f, tq, (6, 7))
                    nc.vector.tensor_copy(fdst[f][b * 4 + tq][:],
                                          hold['pq'][:])
                return [q_first, q_mid((2, 3)), q_mid((4, 5)), q_last]

            def rope_mm(ch, which):
                """RoPE one chunk of q or k, in place"""
                tcl = qtc if which == "q" else ktc
                src = tcl[ch]
                pr = psS.tile([128, 512], F32, name=f"pr{which}{ch}",
                              tag="ps_s")
                nc.tensor.matmul(pr[:], perm_sb[:], src[:],
                                 start=True, stop=True)
                rot = wk.tile([128, 512], BF16, name=f"rot{which}{ch}",
                              tag="rot")
                nc.vector.tensor_mul(rot[:], pr[:],
                                     sin_sb[:, ch * 512:(ch + 1) * 512])
                nc.vector.tensor_mul(src[:], src[:],
                                     cos_sb[:, ch * 512:(ch + 1) * 512])
                nc.vector.tensor_add(src[:], src[:], rot[:])

            def vt_block(kb):
                """V block kb -> [t, d] via bf16 PE transpose"""
                pvt = psS.tile([128, 512], BF16, name=f"pvt{kb}", tag="ps_s")
                nc.tensor.transpose(
                    pvt[:, 0:128],
                    vtc[kb // 4][:, (kb % 4) * 128:(kb % 4 + 1) * 128],
                    id_sb[:])
                eng = nc.scalar if kb < 16 else nc.vector
                if eng is nc.scalar:
                    nc.scalar.copy(v_sb[kb][:, 0:HD], pvt[:, 0:HD])
                    nc.scalar.copy(v_sb[kb][:, HD + 1:2 * HD + 1],
                                   pvt[:, HD:2 * HD])
                else:
                    nc.vector.tensor_copy(v_sb[kb][:, 0:HD], pvt[:, 0:HD])
                    nc.vector.tensor_copy(v_sb[kb][:, HD + 1:2 * HD + 1],
                                          pvt[:, HD:2 * HD])

            def normalize(b, h, qc, avq):
                den = wk.tile([1, 512], F32, name=f"den{b}{h}{qc}", tag="den")
                nc.scalar.copy(den[:], avq[HD:HD + 1, :])
                avs = wk.tile([HD, 512], F32, name=f"avs{b}{h}{qc}",
                              tag="avs")
                nc.vector.tensor_copy(avs[:], avq[0:HD, :])
                # (avs copy stays on DVE: Pool cannot read PSUM)
                recr = wk.tile([1, 512], mybir.dt.float32r,
                               name=f"recr{b}{h}{qc}", tag="recr")
                cst = RECIP_APPROX_FAST_CONSTS
                nc.vector._custom_dve(RECIPROCAL_APPROX_FAST, out=recr[:],
                                      in0=den[:], s0=cst["s0"], s1=cst["s1"],
                                      imm2=cst["imm2"])
                bc = psA.tile([HD, 512], F32, name=f"bc{b}{h}{qc}",
                              tag="ps_a")
                nc.tensor.matmul(bc[:], ones_r[:], recr[:],
                                 start=True, stop=True)
                attn = wk.tile([HD, 512], BF16, name=f"attn{b}{h}{qc}",
                               tag="attn")
                nc.vector.tensor_mul(attn[:], avs[0:HD, :], bc[:])
                # one DMA: [64, (2,256)] -> rows 128qc..128qc+128 of a2a_in
                nc.sync.dma_start(
                    a2a_in[b, h][2 * qc:2 * qc + 2].rearrange(
                        "h p c -> p h c"),
                    attn[:].rearrange("p (h c) -> p h c", h=2))

            def a2a_issue(b, h):
                nc.gpsimd.collective_compute(
                    "AllToAll",
                    mybir.AluOpType.bypass,
                    replica_groups=[list(range(NC))],
                    ins=[a2a_in[b, h][:]],
                    outs=[a2a_out[b, h][:]],
                )

            def att_load(b, h):
                # Pool engine: its stream holds only collectives/memsets, so
                # waiting on the collective blocks nothing else
                nc.gpsimd.dma_start(
                    att_sb[b][HD * h:HD * (h + 1), :].rearrange(
                        "p (c k) -> p c k", c=8),
                    a2a_out[b, h][:].rearrange("c p k -> p c k"))

            ones_b = pp.tile([128, 1], BF16, name="ones_b", tag="ones_b")
            nc.vector.tensor_copy(ones_b[:], ones_f[:])
            dummy_n = [0]

            def dummy_mm():
                """clock-warmer: an M=1 matmul (cheap Ldweights) into a
                never-read psS tile; fills the exp-latency bubble"""
                dummy_n[0] += 1
                d = psS.tile([1, 512], F32, name=f"dm{dummy_n[0]}",
                             tag="ps_s")
                nc.tensor.matmul(d[:], ones_b[:],
                                 xt_big[0][:, 0:512], start=True, stop=True)

            def op_mms(po, b, tb, j, cs):
                for c in cs:
                    nc.tensor.matmul(
                        po[:],
                        att_sb[b][:, 256 * c + 128 * tb:
                                  256 * c + 128 * tb + 128],
                        wp_big[0][:, c * C + j * 512:c * C + j * 512 + 512],
                        start=(c == 0), stop=(c == 7))

            def op_finish(po, b, tb, j):
                ot = wk.tile([128, 512], BF16, name=f"ot{b}{tb}{j}",
                             tag="ot")
                if (tb + j) % 2 == 0:
                    nc.vector.tensor_copy(ot[:], po[:])
                else:
                    nc.scalar.copy(ot[:], po[:])
                eng = nc.sync if (tb * 2 + j) % 2 == 0 else nc.scalar
                eng.dma_start(
                    out_d[b * 256 + tb * 128:b * 256 + (tb + 1) * 128,
                          j * 512:(j + 1) * 512], ot[:])

            def outproj_piece(b, tb, j):
                po = psS.tile([128, 512], F32, name=f"po{b}{tb}{j}",
                              tag="ps_s")
                op_mms(po, b, tb, j, range(8))
                op_finish(po, b, tb, j)

            def outproj_quanta(b, tb, j):
                hold = {}

                def q_first():
                    hold['po'] = psS.tile([128, 512], F32,
                                          name=f"po{b}{tb}{j}", tag="ps_s")
                    op_mms(hold['po'], b, tb, j, (0, 1))

                def q_mid(cs):
                    def g():
                        op_mms(hold['po'], b, tb, j, cs)
                    return g

                def q_last():
                    op_mms(hold['po'], b, tb, j, (6, 7))
                    op_finish(hold['po'], b, tb, j)
                return [q_first, q_mid((2, 3)), q_mid((4, 5)), q_last]

            # ---- attention, qc-outer: one avp accumulator live at a time
            # (psV=2 banks) buys a 5-deep S pipeline (psS=5) so the PE can
            # run ahead of the exp cadence; fillers injected every 2nd kb
            def attention(b, fillers):
                for h in range(HL):
                    hp = h * 64

                    def s_block(kb, qc, b=b, h=h, hp=hp):
                        kch = ktc[b * 4 + kb // 4]
                        koff = (kb % 4) * 128
                        m = kb % 4 if qc == kb // 4 else 0
                        c0 = 128 * m
                        sps = psS.tile([128, 512], F32,
                                       name=f"s{b}{h}{kb}{qc}", tag="ps_s")
                        nc.tensor.matmul(
                            sps[:, c0:512],
                            kch[hp:hp + 64, koff:koff + 128],
                            qtc[b * 4 + qc][hp:hp + 64, c0:512],
                            start=True, stop=True,
                        )
                        pt = ptp.tile([128, 512], BF16,
                                      name=f"pt{b}{h}{kb}{qc}", tag="pt")
                        nc.scalar.activation(
                            pt[:, c0:512], sps[:, c0:512],
                            mybir.ActivationFunctionType.Exp,
                            scale=0.125,
                        )
                        if qc == kb // 4:
                            # Pool engine: empty queue during passes, so the
                            # exp->mask->AV chain never waits behind DVE work
                            nc.gpsimd.tensor_mul(
                                pt[:, c0:512], pt[:, c0:512],
                                mask_big[:, (kb % 4) * 512 + c0:
                                         (kb % 4) * 512 + 512])
                        return pt, c0

                    # descending column order: the pass ends on the short
                    # qc0 column, so the final normalize->staging->trigger
                    # chain rides an uncongested DVE queue
                    for qc in range(3, -1, -1):
                        avq = psV.tile([HD + 1, 512], F32,
                                       name=f"av{b}{h}{qc}", tag="ps_av")
                        nkb = 4 * qc + 4

                        def av_block(kb, pt, c0, avq=avq, b=b, h=h, qc=qc):
                            nc.tensor.matmul(
                                avq[:, c0:512],
                                v_sb[b * 16 + kb][:, h * (HD + 1):
                                                  (h + 1) * (HD + 1)],
                                pt[:, c0:512],
                                start=(kb == 0), stop=(kb == 4 * qc + 3),
                                skip_group_check=bool(c0),
                            )

                        prev = s_block(0, qc)
                        for kb in range(1, nkb):
                            cur = s_block(kb, qc)
                            av_block(kb - 1, *prev)
                            if fillers:
                                fillers.pop(0)()
                            prev = cur
                        av_block(nkb - 1, *prev)
                        normalize(b, h, qc, avq)
                        if fillers:
                            fillers.pop(0)()
                    a2a_issue(b, h)

            # ================= main flow =================
            # batch-0 qkv (q, k), rope b0 threaded through the v slices
            for f in (0, 1):
                for tq in range(4):
                    qkv_piece(0, f, tq)
            rope_mm(0, "q")
            qkv_piece(0, 2, 0)
            rope_mm(0, "k")
            qkv_piece(0, 2, 1)
            rope_mm(1, "q")
            qkv_piece(0, 2, 2)
            rope_mm(1, "k")
            qkv_piece(0, 2, 3)
            rope_mm(2, "q")
            for kb in range(0, 4):
                vt_block(kb)
            rope_mm(2, "k")
            for kb in range(4, 8):
                vt_block(kb)
            rope_mm(3, "q")
            for kb in range(8, 12):
                vt_block(kb)
            rope_mm(3, "k")
            for kb in range(12, 16):
                vt_block(kb)

            # batch-1 x loads go out now, wp behind them
            xt_load(1)
            wp_load()

            # fillers for attention(0): batch-1 qkv + rope + V transposes,
            # as ~430ns quanta sized to the per-block exp bubble
            f0 = []
            for tq in range(4):
                f0 += qkv_quanta(1, 0, tq)
            f0 += qkv_quanta(1, 1, 0)
            f0.append(lambda: rope_mm(4, "q"))
            f0 += qkv_quanta(1, 1, 1)
            f0.append(lambda: rope_mm(5, "q"))
            f0 += qkv_quanta(1, 1, 2)
            f0.append(lambda: rope_mm(6, "q"))
            f0 += qkv_quanta(1, 1, 3)
            f0.append(lambda: rope_mm(7, "q"))
            f0.append(lambda: rope_mm(4, "k"))
            f0 += qkv_quanta(1, 2, 0)
            f0.append(lambda: rope_mm(5, "k"))
            f0.append(lambda: vt_block(16))
            f0.append(lambda: vt_block(17))
            f0 += qkv_quanta(1, 2, 1)
            f0.append(lambda: rope_mm(6, "k"))
            f0.append(lambda: vt_block(18))
            f0.append(lambda: vt_block(19))
            f0 += qkv_quanta(1, 2, 2)
            f0.append(lambda: rope_mm(7, "k"))
            f0.append(lambda: vt_block(20))
            f0.append(lambda: vt_block(21))
            f0 += qkv_quanta(1, 2, 3)
            for kb in range(22, 32):
                f0.append(lambda kb=kb: vt_block(kb))

            attention(0, f0)
            while f0:
                f0.pop(0)()

            att_load(0, 0)
            att_load(0, 1)

            # fillers for attention(1): 1-matmul dummies bridge the exp
            # bubbles; batch-0 outproj quanta go late in head 1 so
            # a2a(0,1) has certainly landed (pops 41-80 are head 1)
            f1 = []
            f1 += [dummy_mm] * 56
            f1 += outproj_quanta(0, 0, 0)
            f1 += outproj_quanta(0, 0, 1)
            f1 += outproj_quanta(0, 1, 0)
            f1 += outproj_quanta(0, 1, 1)
            f1 += [dummy_mm] * 8

            attention(1, f1)
            while f1:
                f1.pop(0)()

            att_load(1, 0)
            att_load(1, 1)
            for tb in range(2):
                for j in range(2):
                    outproj_piece(1, tb, j)

    nc.finalize()
    return nc


def host_inputs(x, w_attn, w_proj):
    x2 = np.ascontiguousarray(x.reshape(BT, C).T).astype(BF)   # [C, BT]

    inv = 1.0 / (ROPE_BASE ** (np.arange(0, HD, 2, dtype=np.float32) / HD))
    tpos = np.arange(T, dtype=np.float32)
    freqs = tpos[:, None] * inv[None, :]                  # [T, 32]
    emb = np.concatenate([freqs, freqs], axis=-1)         # [T, 64]
    cosT = np.cos(emb).T.astype(np.float32)               # [64, T]
    sinT = np.sin(emb).T.astype(np.float32)
    cos_full = np.ascontiguousarray(np.tile(cosT, (2, B))).astype(BF)
    sin_full = np.ascontiguousarray(np.tile(sinT, (2, B))).astype(BF)

    m64 = np.zeros((HD, HD), dtype=np.float32)
    half = HD // 2
    for d in range(half):
        m64[d, d + half] = -1.0
        m64[d + half, d] = 1.0
    perm = np.zeros((128, 128), dtype=np.float32)
    perm[0:HD, 0:HD] = m64
    perm[HD:128, HD:128] = m64
    permT = np.ascontiguousarray(perm.T).astype(BF)

    masks = np.zeros((4, 128, 512), dtype=np.float32)
    qi = np.arange(512)[None, :]
    ki = np.arange(128)[:, None]
    for m in range(4):
        masks[m] = (qi - ki >= m * 128).astype(np.float32)
    masks = masks.astype(BF)

    identB = np.eye(128, dtype=np.float32).astype(BF)
    wpT = np.ascontiguousarray(w_proj.T).astype(BF)       # [c, o]

    in_maps = []
    for i in range(NC):
        r0 = i * (HL * HD)
        wq = w_attn[r0:r0 + HL * HD, :]
        wk_ = w_attn[C + r0:C + r0 + HL * HD, :]
        wv = w_attn[2 * C + r0:2 * C + r0 + HL * HD, :]
        wqkvT = np.ascontiguousarray(
            np.concatenate([wq, wk_, wv], axis=0).T).astype(BF)
        in_maps.append({
            "xT": x2, "wqkvT": wqkvT, "wpT": wpT,
            "cosT": cos_full, "sinT": sin_full, "permT": permT,
            "masks": masks, "identB": identB,
        })
    return in_maps


_NC_CACHE = None


def _get_nc():
    global _NC_CACHE
    if _NC_CACHE is None:
        _NC_CACHE = build()
    return _NC_CACHE


def run(x, w_attn, w_proj, trace=False):
    nc = _get_nc()
    in_maps = host_inputs(np.asarray(x), np.asarray(w_attn),
                          np.asarray(w_proj))
    res = run_bass_kernel_spmd(nc, in_maps, list(range(NC)), trace=trace)
    out = np.empty((B, T, C), dtype=np.float32)
    piece = T // NC
    for i in range(NC):
        sh = np.asarray(res.results[i]["out"]).astype(np.float32)
        out[0, i * piece:(i + 1) * piece] = sh[0:piece]
        out[1, i * piece:(i + 1) * piece] = sh[piece:2 * piece]
    return out, res


def kernel(x, w_attn, w_proj):
    out, _ = run(x, w_attn, w_proj, trace=False)
    return out


# revision 35
# speedup vs baseline: 1.1284x; 1.1284x over previous
"""Causal self-attention with RoPE on 8 TRN2 NeuronCores — v2.

Head-parallel TP as v1 (core i owns heads 2i, 2i+1), redesigned around
the measured v1 bottlenecks:

- One PE stream with qkv/rope/V-transpose/outproj work software-pipelined
  INTO the attention kb-loop as filler, so PE never idles on the Act
  (exp) cadence and the Act engine never waits on phase boundaries.
- All PSUM [128,512] f32 tiles (qkv pq, rope pr, V-transpose, S, outproj
  po) share one 3-buffer pool (psS); avp accumulators keep 4 banks (psV);
  normalize-broadcast keeps 1 (psA).
- cos/sin loaded as TWO whole-tensor DMAs up-front (v1 chunk-loads built
  a 50us credit-semaphore chain that delayed the collectives); xt loads
  merged per (batch, c-block); staging is ONE DMA per q-chunk and
  att_load ONE DMA per (batch, head) via AP rearrange.
- V transposed on PE in bf16 (1 cyc/row), ones columns pre-memset into
  persistent v tiles so AV keeps the fused [65 x 512] denominator-row
  trick.
- PSUM->SBUF casts moved off the Act engine (exp is its critical work)
  onto DVE; big loads issue on the Act HW-DGE queue before exp starts,
  staging/att_load/out on the SP queue.
- Collectives issue on Pool, whose stream contains nothing else.
"""

import numpy as np
import ml_dtypes

import concourse.bass as bass
import concourse.mybir as mybir
import concourse.tile as tile
from concourse import bacc
from concourse.bass_utils import run_bass_kernel_spmd
from concourse.dve_ops import (RECIP_APPROX_FAST_CONSTS,
                               RECIPROCAL_APPROX_FAST)

F32 = mybir.dt.float32
BF16 = mybir.dt.bfloat16

B, T, C = 2, 2048, 1024
H, HD = 16, 64
NC = 8
HL = H // NC          # heads per core = 2
BT = B * T            # 4096
FQKV = 3 * HL * HD    # 384 rows of w_attn per core
TSH = BT // NC        # 512 output rows per core (256 per batch)
NCH = BT // 512       # 8 chunks of 512 t
ROPE_BASE = 10000.0
BF = ml_dtypes.bfloat16


def build():
    nc = bacc.Bacc(None, target_bir_lowering=False)

    xT_d = nc.dram_tensor("xT", [C, BT], BF16, kind="ExternalInput")
    wq_d = nc.dram_tensor("wqkvT", [C, FQKV], BF16, kind="ExternalInput")
    wp_d = nc.dram_tensor("wpT", [C, C], BF16, kind="ExternalInput")
    cos_d = nc.dram_tensor("cosT", [128, BT], BF16, kind="ExternalInput")
    sin_d = nc.dram_tensor("sinT", [128, BT], BF16, kind="ExternalInput")
    perm_d = nc.dram_tensor("permT", [128, 128], BF16, kind="ExternalInput")
    mask_d = nc.dram_tensor("masks", [4, 128, 512], BF16, kind="ExternalInput")
    id_d = nc.dram_tensor("identB", [128, 128], BF16, kind="ExternalInput")
    out_d = nc.dram_tensor("out", [TSH, C], BF16, kind="ExternalOutput")

    # block j of a2a_in = my 128 attention channels for core j's 256 t-rows
    # of batch b; block j of a2a_out = core j's channels for MY 256 t-rows.
    a2a_in = {(b_, h_): nc.dram_tensor(f"a2ain{b_}{h_}", [8, 64, 256], BF16)
              for b_ in range(2) for h_ in range(2)}
    a2a_out = {(b_, h_): nc.dram_tensor(f"a2aout{b_}{h_}", [8, 64, 256], BF16)
               for b_ in range(2) for h_ in range(2)}

    with tile.TileContext(nc) as tc:
        with (
            tc.tile_pool(name="persist", bufs=1) as pp,
            tc.tile_pool(name="work", bufs=4) as wk,
            tc.tile_pool(name="pts", bufs=20) as ptp,
            tc.tile_pool(name="psS", bufs=5, space="PSUM") as psS,
            tc.tile_pool(name="psV", bufs=2, space="PSUM") as psV,
            tc.tile_pool(name="psA", bufs=1, space="PSUM") as psA,
        ):
            # ---- constants / weights: few big DMAs, DRAM side rearranged
            # so all 8 c-blocks land in one SBUF tile per tensor ----
            wq_big = pp.tile([128, 8 * FQKV], BF16, name="wqb", tag="wqb")
            nc.sync.dma_start(
                wq_big[:], wq_d[:].rearrange("(c p) f -> p c f", p=128))
            wq_sb = [wq_big[:, c * FQKV:(c + 1) * FQKV] for c in range(8)]
            id_sb = pp.tile([128, 128], BF16, name="id_sb", tag="id_sb")
            nc.sync.dma_start(id_sb[:], id_d[:])
            perm_sb = pp.tile([128, 128], BF16, name="perm_sb", tag="perm_sb")
            nc.sync.dma_start(perm_sb[:], perm_d[:])
            # xt: [128, 8*2048], one generation per batch; halves split
            # across the two HW-DGE queues so batch 0 lands in ~6us
            xt_big = [None]

            def xt_load(b):
                # 4 DMAs, 2 per HW queue, so the first qkv accumulation
                # chain can chase the c-blocks as they land
                t = pp.tile([128, 8 * 2048], BF16, name=f"xt{b}", tag="xt")
                for q_ in range(4):
                    eng = nc.sync if q_ % 2 == 0 else nc.scalar
                    eng.dma_start(
                        t[:, q_ * 2 * 2048:(q_ + 1) * 2 * 2048],
                        xT_d[q_ * 256:(q_ + 1) * 256,
                             b * 2048:(b + 1) * 2048].rearrange(
                            "(c p) t -> p c t", p=128))
                xt_big[0] = t

            xt_load(0)
            cos_sb = pp.tile([128, BT], BF16, name="cos_sb", tag="cos_sb")
            nc.scalar.dma_start(cos_sb[:], cos_d[:])
            sin_sb = pp.tile([128, BT], BF16, name="sin_sb", tag="sin_sb")
            nc.scalar.dma_start(sin_sb[:], sin_d[:])
            mask_big = pp.tile([128, 4 * 512], BF16, name="maskb", tag="maskb")
            nc.scalar.dma_start(
                mask_big[:], mask_d[:].rearrange("m p c -> p m c"))

            ones_f = pp.tile([128, 1], F32, name="ones_f", tag="ones_f")
            nc.vector.memset(ones_f[:], 1.0)
            ones_r = pp.tile([1, HD], mybir.dt.float32r, name="ones_r",
                             tag="ones_r")
            nc.vector.tensor_copy(ones_r[:],
                                  ones_f[0:1, 0:1].broadcast_to((1, HD)))

            # persistent V tiles [t,d]: cols 64 and 129 stay the memset 1.0
            # (denominator rows of the fused AV); memsets on the idle Pool
            v_sb = []
            for kb in range(32):
                v = pp.tile([128, 2 * (HD + 1)], BF16, name=f"v{kb}",
                            tag=f"v{kb}")
                nc.gpsimd.memset(v[:], 1.0)
                v_sb.append(v)

            qtc = [pp.tile([128, 512], BF16, name=f"qtc{i}", tag=f"qtc{i}")
                   for i in range(NCH)]
            ktc = [pp.tile([128, 512], BF16, name=f"ktc{i}", tag=f"ktc{i}")
                   for i in range(NCH)]
            vtc = [pp.tile([128, 512], BF16, name=f"vtc{i}", tag=f"vtc{i}")
                   for i in range(NCH)]
            fdst = [qtc, ktc, vtc]

            att_sb = {}
            for b in range(2):
                att_sb[b] = pp.tile([128, 2048], BF16, name=f"att{b}",
                                    tag=f"att{b}")
            wp_big = [None]

            def wp_load():
                t = pp.tile([128, 8 * C], BF16, name="wpb", tag="wpb")
                nc.scalar.dma_start(
                    t[:], wp_d[:].rearrange("(c p) o -> p c o", p=128))
                wp_big[0] = t

            # ---- building blocks ----
            def qkv_mms(pq, b, f, tq, cs):
                xt = xt_big[0]
                for c in cs:
                    nc.tensor.matmul(
                        pq[:],
                        wq_big[:, c * FQKV + f * 128:c * FQKV + f * 128 + 128],
                        xt[:, c * 2048 + tq * 512:c * 2048 + tq * 512 + 512],
                        start=(c == 0), stop=(c == 7))

            def qkv_piece(b, f, tq):
                """one [128, 512] slice of the qkv projection for batch b;
                lead-in (b=0) casts go to the then-idle Act engine"""
                pq = psS.tile([128, 512], F32, name=f"pq{b}{f}{tq}",
                              tag="ps_s")
                qkv_mms(pq, b, f, tq, range(8))
                if b == 0:
                    nc.scalar.copy(fdst[f][b * 4 + tq][:], pq[:])
                else:
                    nc.vector.tensor_copy(fdst[f][b * 4 + tq][:], pq[:])

            def qkv_quanta(b, f, tq):
                """same piece as 4 filler quanta of ~2 matmuls (~430ns),
                sized to hide in the exp-latency bubble of one kb block"""
                hold = {}

                def q_first():
                    hold['pq'] = psS.tile([128, 512], F32,
                                          name=f"pq{b}{f}{tq}", tag="ps_s")
                    qkv_mms(hold['pq'], b, f, tq, (0, 1))

                def q_mid(cs):
                    def g():
                        qkv_mms(hold['pq'], b, f, tq, cs)
                    return g

                def q_last():
                    qkv_mms(hold['pq'], b, f, tq, (6, 7))
                    nc.vector.tensor_copy(fdst[f][b * 4 + tq][:],
                                          hold['pq'][:])
                return [q_first, q_mid((2, 3)), q_mid((4, 5)), q_last]

            def rope_mm(ch, which):
                """RoPE one chunk of q or k, in place"""
                tcl = qtc if which == "q" else ktc
                src = tcl[ch]
                pr = psS.tile([128, 512], F32, name=f"pr{which}{ch}",
                              tag="ps_s")
                nc.tensor.matmul(pr[:], perm_sb[:], src[:],
                                 start=True, stop=True)
                rot = wk.tile([128, 512], BF16, name=f"rot{which}{ch}",
                              tag="rot")
                nc.vector.tensor_mul(rot[:], pr[:],
                                     sin_sb[:, ch * 512:(ch + 1) * 512])
                nc.vector.tensor_mul(src[:], src[:],
                                     cos_sb[:, ch * 512:(ch + 1) * 512])
                nc.vector.tensor_add(src[:], src[:], rot[:])

            def vt_block(kb):
                """V block kb -> [t, d] via bf16 PE transpose"""
                pvt = psS.tile([128, 512], BF16, name=f"pvt{kb}", tag="ps_s")
                nc.tensor.transpose(
                    pvt[:, 0:128],
                    vtc[kb // 4][:, (kb % 4) * 128:(kb % 4 + 1) * 128],
                    id_sb[:])
                eng = nc.scalar if kb < 16 else nc.vector
                if eng is nc.scalar:
                    nc.scalar.copy(v_sb[kb][:, 0:HD], pvt[:, 0:HD])
                    nc.scalar.copy(v_sb[kb][:, HD + 1:2 * HD + 1],
                                   pvt[:, HD:2 * HD])
                else:
                    nc.vector.tensor_copy(v_sb[kb][:, 0:HD], pvt[:, 0:HD])
                    nc.vector.tensor_copy(v_sb[kb][:, HD + 1:2 * HD + 1],
                                          pvt[:, HD:2 * HD])

            def normalize(b, h, qc, avq):
                den = wk.tile([1, 512], F32, name=f"den{b}{h}{qc}", tag="den")
                nc.scalar.copy(den[:], avq[HD:HD + 1, :])
                avs = wk.tile([HD, 512], F32, name=f"avs{b}{h}{qc}",
                              tag="avs")
                nc.vector.tensor_copy(avs[:], avq[0:HD, :])
                recr = wk.tile([1, 512], mybir.dt.float32r,
                               name=f"recr{b}{h}{qc}", tag="recr")
                cst = RECIP_APPROX_FAST_CONSTS
                nc.vector._custom_dve(RECIPROCAL_APPROX_FAST, out=recr[:],
                                      in0=den[:], s0=cst["s0"], s1=cst["s1"],
                                      imm2=cst["imm2"])
                bc = psA.tile([HD, 512], F32, name=f"bc{b}{h}{qc}",
                              tag="ps_a")
                nc.tensor.matmul(bc[:], ones_r[:], recr[:],
                                 start=True, stop=True)
                attn = wk.tile([HD, 512], BF16, name=f"attn{b}{h}{qc}",
                               tag="attn")
                nc.vector.tensor_mul(attn[:], avs[0:HD, :], bc[:])
                # one DMA: [64, (2,256)] -> rows 128qc..128qc+128 of a2a_in
                nc.sync.dma_start(
                    a2a_in[b, h][2 * qc:2 * qc + 2].rearrange(
                        "h p c -> p h c"),
                    attn[:].rearrange("p (h c) -> p h c", h=2))

            def a2a_issue(b, h):
                nc.gpsimd.collective_compute(
                    "AllToAll",
                    mybir.AluOpType.bypass,
                    replica_groups=[list(range(NC))],
                    ins=[a2a_in[b, h][:]],
                    outs=[a2a_out[b, h][:]],
                )

            def att_load(b, h):
                # Pool engine: its stream holds only collectives/memsets, so
                # waiting on the collective blocks nothing else
                nc.gpsimd.dma_start(
                    att_sb[b][HD * h:HD * (h + 1), :].rearrange(
                        "p (c k) -> p c k", c=8),
                    a2a_out[b, h][:].rearrange("c p k -> p c k"))

            dummy_n = [0]

            def dummy_mm():
                """clock-warmer: one real-shaped matmul into a never-read
                psS tile; fills the ~300ns exp-latency bubble of a block"""
                dummy_n[0] += 1
                d = psS.tile([128, 512], F32, name=f"dm{dummy_n[0]}",
                             tag="ps_s")
                nc.tensor.matmul(d[:], wq_big[:, 0:128],
                                 xt_big[0][:, 0:512], start=True, stop=True)

            def op_mms(po, b, tb, j, cs):
                for c in cs:
                    nc.tensor.matmul(
                        po[:],
                        att_sb[b][:, 256 * c + 128 * tb:
                                  256 * c + 128 * tb + 128],
                        wp_big[0][:, c * C + j * 512:c * C + j * 512 + 512],
                        start=(c == 0), stop=(c == 7))

            def op_finish(po, b, tb, j):
                ot = wk.tile([128, 512], BF16, name=f"ot{b}{tb}{j}",
                             tag="ot")
                if (tb + j) % 2 == 0:
                    nc.vector.tensor_copy(ot[:], po[:])
                else:
                    nc.scalar.copy(ot[:], po[:])
                eng = nc.sync if (tb * 2 + j) % 2 == 0 else nc.scalar
                eng.dma_start(
                    out_d[b * 256 + tb * 128:b * 256 + (tb + 1) * 128,
                          j * 512:(j + 1) * 512], ot[:])

            def outproj_piece(b, tb, j):
                po = psS.tile([128, 512], F32, name=f"po{b}{tb}{j}",
                              tag="ps_s")
                op_mms(po, b, tb, j, range(8))
                op_finish(po, b, tb, j)

            def outproj_quanta(b, tb, j):
                hold = {}

                def q_first():
                    hold['po'] = psS.tile([128, 512], F32,
                                          name=f"po{b}{tb}{j}", tag="ps_s")
                    op_mms(hold['po'], b, tb, j, (0, 1))

                def q_mid(cs):
                    def g():
                        op_mms(hold['po'], b, tb, j, cs)
                    return g

                def q_last():
                    op_mms(hold['po'], b, tb, j, (6, 7))
                    op_finish(hold['po'], b, tb, j)
                return [q_first, q_mid((2, 3)), q_mid((4, 5)), q_last]

            # ---- attention, qc-outer: one avp accumulator live at a time
            # (psV=2 banks) buys a 5-deep S pipeline (psS=5) so the PE can
            # run ahead of the exp cadence; fillers injected every 2nd kb
            def attention(b, fillers):
                for h in range(HL):
                    hp = h * 64

                    def s_block(kb, qc, b=b, h=h, hp=hp):
                        kch = ktc[b * 4 + kb // 4]
                        koff = (kb % 4) * 128
                        m = kb % 4 if qc == kb // 4 else 0
                        c0 = 128 * m
                        sps = psS.tile([128, 512], F32,
                                       name=f"s{b}{h}{kb}{qc}", tag="ps_s")
                        nc.tensor.matmul(
                            sps[:, c0:512],
                            kch[hp:hp + 64, koff:koff + 128],
                            qtc[b * 4 + qc][hp:hp + 64, c0:512],
                            start=True, stop=True,
                        )
                        pt = ptp.tile([128, 512], BF16,
                                      name=f"pt{b}{h}{kb}{qc}", tag="pt")
                        nc.scalar.activation(
                            pt[:, c0:512], sps[:, c0:512],
                            mybir.ActivationFunctionType.Exp,
                            scale=0.125,
                        )
                        if qc == kb // 4:
                            nc.vector.tensor_mul(
                                pt[:, c0:512], pt[:, c0:512],
                                mask_big[:, (kb % 4) * 512 + c0:
                                         (kb % 4) * 512 + 512])
                        return pt, c0

                    # descending: the pass ends on the short qc0 column, so
                    # the last normalize->staging->collective-trigger chain
                    # rides an uncongested DVE queue
                    for qc in range(3, -1, -1):
                        avq = psV.tile([HD + 1, 512], F32,
                                       name=f"av{b}{h}{qc}", tag="ps_av")
                        nkb = 4 * qc + 4

                        def av_block(kb, pt, c0, avq=avq, b=b, h=h, qc=qc):
                            nc.tensor.matmul(
                                avq[:, c0:512],
                                v_sb[b * 16 + kb][:, h * (HD + 1):
                                                  (h + 1) * (HD + 1)],
                                pt[:, c0:512],
                                start=(kb == 0), stop=(kb == 4 * qc + 3),
                                skip_group_check=bool(c0),
                            )

                        prev = s_block(0, qc)
                        for kb in range(1, nkb):
                            cur = s_block(kb, qc)
                            av_block(kb - 1, *prev)
                            if fillers:
                                fillers.pop(0)()
                            prev = cur
                        av_block(nkb - 1, *prev)
                        normalize(b, h, qc, avq)
                        if fillers:
                            fillers.pop(0)()
                    a2a_issue(b, h)

            # ================= main flow =================
            # batch-0 qkv (q, k), rope b0 threaded through the v slices
            for f in (0, 1):
                for tq in range(4):
                    qkv_piece(0, f, tq)
            rope_mm(0, "q")
            qkv_piece(0, 2, 0)
            rope_mm(0, "k")
            qkv_piece(0, 2, 1)
            rope_mm(1, "q")
            qkv_piece(0, 2, 2)
            rope_mm(1, "k")
            qkv_piece(0, 2, 3)
            rope_mm(2, "q")
            for kb in range(0, 4):
                vt_block(kb)
            rope_mm(2, "k")
            for kb in range(4, 8):
                vt_block(kb)
            rope_mm(3, "q")
            for kb in range(8, 12):
                vt_block(kb)
            rope_mm(3, "k")
            for kb in range(12, 16):
                vt_block(kb)

            # batch-1 x loads go out now, wp behind them
            xt_load(1)
            wp_load()

            # fillers for attention(0): batch-1 qkv + rope + V transposes,
            # as ~430ns quanta sized to the per-block exp bubble
            f0 = []
            for tq in range(4):
                f0 += qkv_quanta(1, 0, tq)
            f0 += qkv_quanta(1, 1, 0)
            f0.append(lambda: rope_mm(4, "q"))
            f0 += qkv_quanta(1, 1, 1)
            f0.append(lambda: rope_mm(5, "q"))
            f0 += qkv_quanta(1, 1, 2)
            f0.append(lambda: rope_mm(6, "q"))
            f0 += qkv_quanta(1, 1, 3)
            f0.append(lambda: rope_mm(7, "q"))
            f0.append(lambda: rope_mm(4, "k"))
            f0 += qkv_quanta(1, 2, 0)
            f0.append(lambda: rope_mm(5, "k"))
            f0.append(lambda: vt_block(16))
            f0.append(lambda: vt_block(17))
            f0 += qkv_quanta(1, 2, 1)
            f0.append(lambda: rope_mm(6, "k"))
            f0.append(lambda: vt_block(18))
            f0.append(lambda: vt_block(19))
            f0 += qkv_quanta(1, 2, 2)
            f0.append(lambda: rope_mm(7, "k"))
            f0.append(lambda: vt_block(20))
            f0.append(lambda: vt_block(21))
            f0 += qkv_quanta(1, 2, 3)
            for kb in range(22, 32):
                f0.append(lambda kb=kb: vt_block(kb))

            attention(0, f0)
            while f0:
                f0.pop(0)()

            att_load(0, 0)
            att_load(0, 1)

            # fillers for attention(1): 1-matmul dummies bridge the exp
            # bubbles; batch-0 outproj quanta go late in head 1 so
            # a2a(0,1) has certainly landed (pops 41-80 are head 1)
            f1 = []
            f1 += [dummy_mm] * 56
            f1 += outproj_quanta(0, 0, 0)
            f1 += outproj_quanta(0, 0, 1)
            f1 += outproj_quanta(0, 1, 0)
            f1 += outproj_quanta(0, 1, 1)
            f1 += [dummy_mm] * 8

            attention(1, f1)
            while f1:
                f1.pop(0)()

            att_load(1, 0)
            att_load(1, 1)
            for tb in range(2):
                for j in range(2):
                    outproj_piece(1, tb, j)

    nc.finalize()
    return nc


def host_inputs(x, w_attn, w_proj):
    x2 = np.ascontiguousarray(x.reshape(BT, C).T).astype(BF)   # [C, BT]

    inv = 1.0 / (ROPE_BASE ** (np.arange(0, HD, 2, dtype=np.float32) / HD))
    tpos = np.arange(T, dtype=np.float32)
    freqs = tpos[:, None] * inv[None, :]                  # [T, 32]
    emb = np.concatenate([freqs, freqs], axis=-1)         # [T, 64]
    cosT = np.cos(emb).T.astype(np.float32)               # [64, T]
    sinT = np.sin(emb).T.astype(np.float32)
    cos_full = np.ascontiguousarray(np.tile(cosT, (2, B))).astype(BF)
    sin_full = np.ascontiguousarray(np.tile(sinT, (2, B))).astype(BF)

    m64 = np.zeros((HD, HD), dtype=np.float32)
    half = HD // 2
    for d in range(half):
        m64[d, d + half] = -1.0
        m64[d + half, d] = 1.0
    perm = np.zeros((128, 128), dtype=np.float32)
    perm[0:HD, 0:HD] = m64
    perm[HD:128, HD:128] = m64
    permT = np.ascontiguousarray(perm.T).astype(BF)

    masks = np.zeros((4, 128, 512), dtype=np.float32)
    qi = np.arange(512)[None, :]
    ki = np.arange(128)[:, None]
    for m in range(4):
        masks[m] = (qi - ki >= m * 128).astype(np.float32)
    masks = masks.astype(BF)

    identB = np.eye(128, dtype=np.float32).astype(BF)
    wpT = np.ascontiguousarray(w_proj.T).astype(BF)       # [c, o]

    in_maps = []
    for i in range(NC):
        r0 = i * (HL * HD)
        wq = w_attn[r0:r0 + HL * HD, :]
        wk_ = w_attn[C + r0:C + r0 + HL * HD, :]
        wv = w_attn[2 * C + r0:2 * C + r0 + HL * HD, :]
        wqkvT = np.ascontiguousarray(
            np.concatenate([wq, wk_, wv], axis=0).T).astype(BF)
        in_maps.append({
            "xT": x2, "wqkvT": wqkvT, "wpT": wpT,
            "cosT": cos_full, "sinT": sin_full, "permT": permT,
            "masks": masks, "identB": identB,
        })
    return in_maps


_NC_CACHE = None


def _get_nc():
    global _NC_CACHE
    if _NC_CACHE is None:
        _NC_CACHE = build()
    return _NC_CACHE


def run(x, w_attn, w_proj, trace=False):
    nc = _get_nc()
    in_maps = host_inputs(np.asarray(x), np.asarray(w_attn),
                          np.asarray(w_proj))
    res = run_bass_kernel_spmd(nc, in_maps, list(range(NC)), trace=trace)
    out = np.empty((B, T, C), dtype=np.float32)
    piece = T // NC
    for i in range(NC):
        sh = np.asarray(res.results[i]["out"]).astype(np.float32)
        out[0, i * piece:(i + 1) * piece] = sh[0:piece]
        out[1, i * piece:(i + 1) * piece] = sh[piece:2 * piece]
    return out, res


def kernel(x, w_attn, w_proj):
    out, _ = run(x, w_attn, w_proj, trace=False)
    return out


# revision 40
# speedup vs baseline: 1.2581x; 1.1150x over previous
"""Causal self-attention with RoPE on 8 TRN2 NeuronCores — v2.

Head-parallel TP as v1 (core i owns heads 2i, 2i+1), redesigned around
the measured v1 bottlenecks:

- One PE stream with qkv/rope/V-transpose/outproj work software-pipelined
  INTO the attention kb-loop as filler, so PE never idles on the Act
  (exp) cadence and the Act engine never waits on phase boundaries.
- All PSUM [128,512] f32 tiles (qkv pq, rope pr, V-transpose, S, outproj
  po) share one 3-buffer pool (psS); avp accumulators keep 4 banks (psV);
  normalize-broadcast keeps 1 (psA).
- cos/sin loaded as TWO whole-tensor DMAs up-front (v1 chunk-loads built
  a 50us credit-semaphore chain that delayed the collectives); xt loads
  merged per (batch, c-block); staging is ONE DMA per q-chunk and
  att_load ONE DMA per (batch, head) via AP rearrange.
- V transposed on PE in bf16 (1 cyc/row), ones columns pre-memset into
  persistent v tiles so AV keeps the fused [65 x 512] denominator-row
  trick.
- PSUM->SBUF casts moved off the Act engine (exp is its critical work)
  onto DVE; big loads issue on the Act HW-DGE queue before exp starts,
  staging/att_load/out on the SP queue.
- Collectives issue on Pool, whose stream contains nothing else.
"""

import numpy as np
import ml_dtypes

import concourse.bass as bass
import concourse.mybir as mybir
import concourse.tile as tile
from concourse import bacc
from concourse.bass_utils import run_bass_kernel_spmd
from concourse.dve_ops import (RECIP_APPROX_FAST_CONSTS,
                               RECIPROCAL_APPROX_FAST)

F32 = mybir.dt.float32
BF16 = mybir.dt.bfloat16

B, T, C = 2, 2048, 1024
H, HD = 16, 64
NC = 8
HL = H // NC          # heads per core = 2
BT = B * T            # 4096
FQKV = 3 * HL * HD    # 384 rows of w_attn per core
TSH = BT // NC        # 512 output rows per core (256 per batch)
NCH = BT // 512       # 8 chunks of 512 t
ROPE_BASE = 10000.0
BF = ml_dtypes.bfloat16


def build():
    nc = bacc.Bacc(None, target_bir_lowering=False)

    xT_d = nc.dram_tensor("xT", [C, BT], BF16, kind="ExternalInput")
    wq_d = nc.dram_tensor("wqkvT", [C, FQKV], BF16, kind="ExternalInput")
    wp_d = nc.dram_tensor("wpT", [C, C], BF16, kind="ExternalInput")
    cos_d = nc.dram_tensor("cosT", [128, BT], BF16, kind="ExternalInput")
    sin_d = nc.dram_tensor("sinT", [128, BT], BF16, kind="ExternalInput")
    perm_d = nc.dram_tensor("permT", [128, 128], BF16, kind="ExternalInput")
    mask_d = nc.dram_tensor("masks", [4, 128, 512], BF16, kind="ExternalInput")
    id_d = nc.dram_tensor("identB", [128, 128], BF16, kind="ExternalInput")
    out_d = nc.dram_tensor("out", [TSH, C], BF16, kind="ExternalOutput")

    # block j of a2a_in = my 128 attention channels for core j's 256 t-rows
    # of batch b; block j of a2a_out = core j's channels for MY 256 t-rows.
    a2a_in = {(b_, h_): nc.dram_tensor(f"a2ain{b_}{h_}", [8, 64, 256], BF16)
              for b_ in range(2) for h_ in range(2)}
    a2a_out = {(b_, h_): nc.dram_tensor(f"a2aout{b_}{h_}", [8, 64, 256], BF16)
               for b_ in range(2) for h_ in range(2)}

    with tile.TileContext(nc) as tc:
        with (
            tc.tile_pool(name="persist", bufs=1) as pp,
            tc.tile_pool(name="work", bufs=4) as wk,
            tc.tile_pool(name="pts", bufs=20) as ptp,
            tc.tile_pool(name="psS", bufs=5, space="PSUM") as psS,
            tc.tile_pool(name="psV", bufs=2, space="PSUM") as psV,
            tc.tile_pool(name="psA", bufs=1, space="PSUM") as psA,
        ):
            # ---- constants / weights: few big DMAs, DRAM side rearranged
            # so all 8 c-blocks land in one SBUF tile per tensor ----
            wq_big = pp.tile([128, 8 * FQKV], BF16, name="wqb", tag="wqb")
            nc.sync.dma_start(
                wq_big[:, 0:4 * FQKV],
                wq_d[0:512].rearrange("(c p) f -> p c f", p=128))
            nc.scalar.dma_start(
                wq_big[:, 4 * FQKV:],
                wq_d[512:1024].rearrange("(c p) f -> p c f", p=128))
            id_sb = pp.tile([128, 128], BF16, name="id_sb", tag="id_sb")
            nc.sync.dma_start(id_sb[:], id_d[:])
            perm_sb = pp.tile([128, 128], BF16, name="perm_sb", tag="perm_sb")
            nc.sync.dma_start(perm_sb[:], perm_d[:])
            # xt: [128, 8*2048], one generation per batch; halves split
            # across the two HW-DGE queues so batch 0 lands in ~6us
            xt_big = [None]

            def xt_load(b):
                # 4 DMAs, 2 per HW queue, so the first qkv accumulation
                # chain can chase the c-blocks as they land
                t = pp.tile([128, 8 * 2048], BF16, name=f"xt{b}", tag="xt")
                for q_ in range(4):
                    eng = nc.sync if q_ % 2 == 0 else nc.scalar
                    eng.dma_start(
                        t[:, q_ * 2 * 2048:(q_ + 1) * 2 * 2048],
                        xT_d[q_ * 256:(q_ + 1) * 256,
                             b * 2048:(b + 1) * 2048].rearrange(
                            "(c p) t -> p c t", p=128))
                xt_big[0] = t

            xt_load(0)
            cos_sb = pp.tile([128, BT], BF16, name="cos_sb", tag="cos_sb")
            nc.scalar.dma_start(cos_sb[:], cos_d[:])
            sin_sb = pp.tile([128, BT], BF16, name="sin_sb", tag="sin_sb")
            nc.scalar.dma_start(sin_sb[:], sin_d[:])
            mask_big = pp.tile([128, 4 * 512], BF16, name="maskb", tag="maskb")
            nc.scalar.dma_start(
                mask_big[:], mask_d[:].rearrange("m p c -> p m c"))

            ones_f = pp.tile([128, 1], F32, name="ones_f", tag="ones_f")
            nc.vector.memset(ones_f[:], 1.0)
            ones_r = pp.tile([1, HD], mybir.dt.float32r, name="ones_r",
                             tag="ones_r")
            nc.vector.tensor_copy(ones_r[:],
                                  ones_f[0:1, 0:1].broadcast_to((1, HD)))

            # persistent V tiles [t,d]: cols 64 and 129 stay the memset 1.0
            # (denominator rows of the fused AV); memsets on the idle Pool
            v_sb = []
            for kb in range(32):
                v = pp.tile([128, 2 * (HD + 1)], BF16, name=f"v{kb}",
                            tag=f"v{kb}")
                nc.gpsimd.memset(v[:], 1.0)
                v_sb.append(v)

            qtc = [pp.tile([128, 512], BF16, name=f"qtc{i}", tag=f"qtc{i}")
                   for i in range(NCH)]
            ktc = [pp.tile([128, 512], BF16, name=f"ktc{i}", tag=f"ktc{i}")
                   for i in range(NCH)]
            vtc = [pp.tile([128, 512], BF16, name=f"vtc{i}", tag=f"vtc{i}")
                   for i in range(NCH)]
            fdst = [qtc, ktc, vtc]

            att_sb = {}
            for b in range(2):
                att_sb[b] = pp.tile([128, 2048], BF16, name=f"att{b}",
                                    tag=f"att{b}")
            wp_big = [None]

            def wp_load():
                t = pp.tile([128, 8 * C], BF16, name="wpb", tag="wpb")
                nc.scalar.dma_start(
                    t[:], wp_d[:].rearrange("(c p) o -> p c o", p=128))
                wp_big[0] = t

            # ---- building blocks ----
            def qkv_mms(pq, b, f, tq, cs):
                xt = xt_big[0]
                for c in cs:
                    nc.tensor.matmul(
                        pq[:],
                        wq_big[:, c * FQKV + f * 128:c * FQKV + f * 128 + 128],
                        xt[:, c * 2048 + tq * 512:c * 2048 + tq * 512 + 512],
                        start=(c == 0), stop=(c == 7))

            def qkv_piece(b, f, tq):
                """one [128, 512] slice of the qkv projection for batch b;
                lead-in (b=0) casts go to the then-idle Act engine"""
                pq = psS.tile([128, 512], F32, name=f"pq{b}{f}{tq}",
                              tag="ps_s")
                qkv_mms(pq, b, f, tq, range(8))
                if b == 0:
                    nc.scalar.copy(fdst[f][b * 4 + tq][:], pq[:])
                else:
                    nc.vector.tensor_copy(fdst[f][b * 4 + tq][:], pq[:])

            def qkv_quanta(b, f, tq):
                """same piece as 4 filler quanta of ~2 matmuls (~430ns),
                sized to hide in the exp-latency bubble of one kb block"""
                hold = {}

                def q_first():
                    hold['pq'] = psS.tile([128, 512], F32,
                                          name=f"pq{b}{f}{tq}", tag="ps_s")
                    qkv_mms(hold['pq'], b, f, tq, (0, 1))

                def q_mid(cs):
                    def g():
                        qkv_mms(hold['pq'], b, f, tq, cs)
                    return g

                def q_last():
                    qkv_mms(hold['pq'], b, f, tq, (6, 7))
                    nc.vector.tensor_copy(fdst[f][b * 4 + tq][:],
                                          hold['pq'][:])
                return [q_first, q_mid((2, 3)), q_mid((4, 5)), q_last]

            def rope_mm(ch, which):
                """RoPE one chunk of q or k, in place"""
                tcl = qtc if which == "q" else ktc
                src = tcl[ch]
                pr = psS.tile([128, 512], F32, name=f"pr{which}{ch}",
                              tag="ps_s")
                nc.tensor.matmul(pr[:], perm_sb[:], src[:],
                                 start=True, stop=True)
                rot = wk.tile([128, 512], BF16, name=f"rot{which}{ch}",
                              tag="rot")
                nc.vector.tensor_mul(rot[:], pr[:],
                                     sin_sb[:, ch * 512:(ch + 1) * 512])
                nc.vector.tensor_mul(src[:], src[:],
                                     cos_sb[:, ch * 512:(ch + 1) * 512])
                nc.vector.tensor_add(src[:], src[:], rot[:])

            def vt_block(kb):
                """V block kb -> [t, d] via bf16 PE transpose"""
                pvt = psS.tile([128, 512], BF16, name=f"pvt{kb}", tag="ps_s")
                nc.tensor.transpose(
                    pvt[:, 0:128],
                    vtc[kb // 4][:, (kb % 4) * 128:(kb % 4 + 1) * 128],
                    id_sb[:])
                eng = nc.scalar if kb < 16 else nc.vector
                if eng is nc.scalar:
                    nc.scalar.copy(v_sb[kb][:, 0:HD], pvt[:, 0:HD])
                    nc.scalar.copy(v_sb[kb][:, HD + 1:2 * HD + 1],
                                   pvt[:, HD:2 * HD])
                else:
                    nc.vector.tensor_copy(v_sb[kb][:, 0:HD], pvt[:, 0:HD])
                    nc.vector.tensor_copy(v_sb[kb][:, HD + 1:2 * HD + 1],
                                          pvt[:, HD:2 * HD])

            def normalize(b, h, qc, avq):
                den = wk.tile([1, 512], F32, name=f"den{b}{h}{qc}", tag="den")
                nc.vector.tensor_copy(den[:], avq[HD:HD + 1, :])
                avs = wk.tile([HD, 512], F32, name=f"avs{b}{h}{qc}",
                              tag="avs")
                nc.vector.tensor_copy(avs[:], avq[0:HD, :])
                recr = wk.tile([1, 512], mybir.dt.float32r,
                               name=f"recr{b}{h}{qc}", tag="recr")
                cst = RECIP_APPROX_FAST_CONSTS
                nc.vector._custom_dve(RECIPROCAL_APPROX_FAST, out=recr[:],
                                      in0=den[:], s0=cst["s0"], s1=cst["s1"],
                                      imm2=cst["imm2"])
                bc = psA.tile([HD, 512], F32, name=f"bc{b}{h}{qc}",
                              tag="ps_a")
                nc.tensor.matmul(bc[:], ones_r[:], recr[:],
                                 start=True, stop=True)
                attn = wk.tile([HD, 512], BF16, name=f"attn{b}{h}{qc}",
                               tag="attn")
                nc.vector.tensor_mul(attn[:], avs[0:HD, :], bc[:])
                # one DMA: [64, (2,256)] -> rows 128qc..128qc+128 of a2a_in
                nc.sync.dma_start(
                    a2a_in[b, h][2 * qc:2 * qc + 2].rearrange(
                        "h p c -> p h c"),
                    attn[:].rearrange("p (h c) -> p h c", h=2))

            def a2a_issue(b, h):
                nc.gpsimd.collective_compute(
                    "AllToAll",
                    mybir.AluOpType.bypass,
                    replica_groups=[list(range(NC))],
                    ins=[a2a_in[b, h][:]],
                    outs=[a2a_out[b, h][:]],
                )

            def att_load(b, h):
                # Pool engine: its stream holds only collectives/memsets, so
                # waiting on the collective blocks nothing else
                nc.gpsimd.dma_start(
                    att_sb[b][HD * h:HD * (h + 1), :].rearrange(
                        "p (c k) -> p c k", c=8),
                    a2a_out[b, h][:].rearrange("c p k -> p c k"))

            dummy_n = [0]

            def dummy_mm():
                """clock-warmer: one real-shaped matmul into a never-read
                psS tile; fills the ~300ns exp-latency bubble of a block"""
                dummy_n[0] += 1
                d = psS.tile([128, 512], F32, name=f"dm{dummy_n[0]}",
                             tag="ps_s")
                nc.tensor.matmul(d[:], wq_big[:, 0:128],
                                 xt_big[0][:, 0:512], start=True, stop=True)

            def op_mms(po, b, tb, j, cs):
                for c in cs:
                    nc.tensor.matmul(
                        po[:],
                        att_sb[b][:, 256 * c + 128 * tb:
                                  256 * c + 128 * tb + 128],
                        wp_big[0][:, c * C + j * 512:c * C + j * 512 + 512],
                        start=(c == 0), stop=(c == 7))

            def op_finish(po, b, tb, j):
                # keep the Act engine free for exp: casts on DVE, DMA
                # issues on the SP queue
                ot = wk.tile([128, 512], BF16, name=f"ot{b}{tb}{j}",
                             tag="ot")
                nc.vector.tensor_copy(ot[:], po[:])
                nc.sync.dma_start(
                    out_d[b * 256 + tb * 128:b * 256 + (tb + 1) * 128,
                          j * 512:(j + 1) * 512], ot[:])

            def outproj_piece(b, tb, j):
                po = psS.tile([128, 512], F32, name=f"po{b}{tb}{j}",
                              tag="ps_s")
                op_mms(po, b, tb, j, range(8))
                op_finish(po, b, tb, j)

            def outproj_quanta(b, tb, j):
                hold = {}

                def q_first():
                    hold['po'] = psS.tile([128, 512], F32,
                                          name=f"po{b}{tb}{j}", tag="ps_s")
                    op_mms(hold['po'], b, tb, j, (0, 1))

                def q_mid(cs):
                    def g():
                        op_mms(hold['po'], b, tb, j, cs)
                    return g

                def q_last():
                    op_mms(hold['po'], b, tb, j, (6, 7))
                    op_finish(hold['po'], b, tb, j)
                return [q_first, q_mid((2, 3)), q_mid((4, 5)), q_last]

            # ---- attention, qc-outer: one avp accumulator live at a time
            # (psV=2 banks) buys a 5-deep S pipeline (psS=5) so the PE can
            # run ahead of the exp cadence; fillers injected every 2nd kb
            def attention(b, fillers):
                for h in range(HL):
                    hp = h * 64

                    def s_block(kb, qc, b=b, h=h, hp=hp):
                        kch = ktc[b * 4 + kb // 4]
                        koff = (kb % 4) * 128
                        m = kb % 4 if qc == kb // 4 else 0
                        c0 = 128 * m
                        sps = psS.tile([128, 512], F32,
                                       name=f"s{b}{h}{kb}{qc}", tag="ps_s")
                        nc.tensor.matmul(
                            sps[:, c0:512],
                            kch[hp:hp + 64, koff:koff + 128],
                            qtc[b * 4 + qc][hp:hp + 64, c0:512],
                            start=True, stop=True,
                        )
                        pt = ptp.tile([128, 512], BF16,
                                      name=f"pt{b}{h}{kb}{qc}", tag="pt")
                        nc.scalar.activation(
                            pt[:, c0:512], sps[:, c0:512],
                            mybir.ActivationFunctionType.Exp,
                            scale=0.125,
                        )
                        if qc == kb // 4:
                            nc.vector.tensor_mul(
                                pt[:, c0:512], pt[:, c0:512],
                                mask_big[:, (kb % 4) * 512 + c0:
                                         (kb % 4) * 512 + 512])
                        return pt, c0

                    for qc in range(4):
                        avq = psV.tile([HD + 1, 512], F32,
                                       name=f"av{b}{h}{qc}", tag="ps_av")
                        nkb = 4 * qc + 4

                        def av_block(kb, pt, c0, avq=avq, b=b, h=h, qc=qc):
                            nc.tensor.matmul(
                                avq[:, c0:512],
                                v_sb[b * 16 + kb][:, h * (HD + 1):
                                                  (h + 1) * (HD + 1)],
                                pt[:, c0:512],
                                start=(kb == 0), stop=(kb == 4 * qc + 3),
                                skip_group_check=bool(c0),
                            )

                        prev = s_block(0, qc)
                        for kb in range(1, nkb):
                            cur = s_block(kb, qc)
                            av_block(kb - 1, *prev)
                            if fillers:
                                fillers.pop(0)()
                            prev = cur
                        av_block(nkb - 1, *prev)
                        normalize(b, h, qc, avq)
                        if fillers:
                            fillers.pop(0)()
                    a2a_issue(b, h)

            # ================= main flow =================
            # batch-0 qkv (q, k), rope b0 threaded through the v slices
            for f in (0, 1):
                for tq in range(4):
                    qkv_piece(0, f, tq)
            rope_mm(0, "q")
            qkv_piece(0, 2, 0)
            rope_mm(0, "k")
            qkv_piece(0, 2, 1)
            rope_mm(1, "q")
            qkv_piece(0, 2, 2)
            rope_mm(1, "k")
            qkv_piece(0, 2, 3)
            rope_mm(2, "q")
            for kb in range(0, 4):
                vt_block(kb)
            rope_mm(2, "k")
            for kb in range(4, 8):
                vt_block(kb)
            rope_mm(3, "q")
            for kb in range(8, 12):
                vt_block(kb)
            rope_mm(3, "k")
            for kb in range(12, 16):
                vt_block(kb)

            # batch-1 x loads go out now, wp behind them
            xt_load(1)
            wp_load()

            # fillers for attention(0): batch-1 qkv + rope + V transposes,
            # as ~430ns quanta sized to the per-block exp bubble
            f0 = []
            for tq in range(4):
                f0 += qkv_quanta(1, 0, tq)
            f0 += qkv_quanta(1, 1, 0)
            f0.append(lambda: rope_mm(4, "q"))
            f0 += qkv_quanta(1, 1, 1)
            f0.append(lambda: rope_mm(5, "q"))
            f0 += qkv_quanta(1, 1, 2)
            f0.append(lambda: rope_mm(6, "q"))
            f0 += qkv_quanta(1, 1, 3)
            f0.append(lambda: rope_mm(7, "q"))
            f0.append(lambda: rope_mm(4, "k"))
            f0 += qkv_quanta(1, 2, 0)
            f0.append(lambda: rope_mm(5, "k"))
            f0.append(lambda: vt_block(16))
            f0.append(lambda: vt_block(17))
            f0 += qkv_quanta(1, 2, 1)
            f0.append(lambda: rope_mm(6, "k"))
            f0.append(lambda: vt_block(18))
            f0.append(lambda: vt_block(19))
            f0 += qkv_quanta(1, 2, 2)
            f0.append(lambda: rope_mm(7, "k"))
            f0.append(lambda: vt_block(20))
            f0.append(lambda: vt_block(21))
            f0 += qkv_quanta(1, 2, 3)
            for kb in range(22, 32):
                f0.append(lambda kb=kb: vt_block(kb))

            attention(0, f0)
            while f0:
                f0.pop(0)()

            att_load(0, 0)
            att_load(0, 1)

            # fillers for attention(1): 1-matmul dummies bridge the exp
            # bubbles; batch-0 outproj quanta go late in head 1 so
            # a2a(0,1) has certainly landed (pops 41-80 are head 1)
            # dummies every OTHER block: every-slot dummies make the PE
            # stream (~1030ns/block incl. Ldweights) overtake the 686ns
            # exp cadence; bare blocks let the clock decay. Alternate.
            noop = lambda: None
            f1 = []
            for _ in range(28):
                f1 += [dummy_mm, noop]
            f1 += outproj_quanta(0, 0, 0)
            f1 += outproj_quanta(0, 0, 1)
            f1 += outproj_quanta(0, 1, 0)
            f1 += outproj_quanta(0, 1, 1)
            f1 += [dummy_mm, noop] * 4

            attention(1, f1)
            while f1:
                f1.pop(0)()

            att_load(1, 0)
            att_load(1, 1)
            for tb in range(2):
                for j in range(2):
                    outproj_piece(1, tb, j)

    nc.finalize()
    return nc


def host_inputs(x, w_attn, w_proj):
    x2 = np.ascontiguousarray(x.reshape(BT, C).T).astype(BF)   # [C, BT]

    inv = 1.0 / (ROPE_BASE ** (np.arange(0, HD, 2, dtype=np.float32) / HD))
    tpos = np.arange(T, dtype=np.float32)
    freqs = tpos[:, None] * inv[None, :]                  # [T, 32]
    emb = np.concatenate([freqs, freqs], axis=-1)         # [T, 64]
    cosT = np.cos(emb).T.astype(np.float32)               # [64, T]
    sinT = np.sin(emb).T.astype(np.float32)
    cos_full = np.ascontiguousarray(np.tile(cosT, (2, B))).astype(BF)
    sin_full = np.ascontiguousarray(np.tile(sinT, (2, B))).astype(BF)

    m64 = np.zeros((HD, HD), dtype=np.float32)
    half = HD // 2
    for d in range(half):
        m64[d, d + half] = -1.0
        m64[d + half, d] = 1.0
    perm = np.zeros((128, 128), dtype=np.float32)
    perm[0:HD, 0:HD] = m64
    perm[HD:128, HD:128] = m64
    permT = np.ascontiguousarray(perm.T).astype(BF)

    masks = np.zeros((4, 128, 512), dtype=np.float32)
    qi = np.arange(512)[None, :]
    ki = np.arange(128)[:, None]
    for m in range(4):
        masks[m] = (qi - ki >= m * 128).astype(np.float32)
    masks = masks.astype(BF)

    identB = np.eye(128, dtype=np.float32).astype(BF)
    wpT = np.ascontiguousarray(w_proj.T).astype(BF)       # [c, o]

    in_maps = []
    for i in range(NC):
        r0 = i * (HL * HD)
        wq = w_attn[r0:r0 + HL * HD, :]
        wk_ = w_attn[C + r0:C + r0 + HL * HD, :]
        wv = w_attn[2 * C + r0:2 * C + r0 + HL * HD, :]
        wqkvT = np.ascontiguousarray(
            np.concatenate([wq, wk_, wv], axis=0).T).astype(BF)
        in_maps.append({
            "xT": x2, "wqkvT": wqkvT, "wpT": wpT,
            "cosT": cos_full, "sinT": sin_full, "permT": permT,
            "masks": masks, "identB": identB,
        })
    return in_maps


_NC_CACHE = None


def _get_nc():
    global _NC_CACHE
    if _NC_CACHE is None:
        _NC_CACHE = build()
    return _NC_CACHE


def run(x, w_attn, w_proj, trace=False):
    nc = _get_nc()
    in_maps = host_inputs(np.asarray(x), np.asarray(w_attn),
                          np.asarray(w_proj))
    res = run_bass_kernel_spmd(nc, in_maps, list(range(NC)), trace=trace)
    out = np.empty((B, T, C), dtype=np.float32)
    piece = T // NC
    for i in range(NC):
        sh = np.asarray(res.results[i]["out"]).astype(np.float32)
        out[0, i * piece:(i + 1) * piece] = sh[0:piece]
        out[1, i * piece:(i + 1) * piece] = sh[piece:2 * piece]
    return out, res


def kernel(x, w_attn, w_proj):
    out, _ = run(x, w_attn, w_proj, trace=False)
    return out


# revision 41
# speedup vs baseline: 1.3293x; 1.0565x over previous
"""Causal self-attention with RoPE on 8 TRN2 NeuronCores — v2.

Head-parallel TP as v1 (core i owns heads 2i, 2i+1), redesigned around
the measured v1 bottlenecks:

- One PE stream with qkv/rope/V-transpose/outproj work software-pipelined
  INTO the attention kb-loop as filler, so PE never idles on the Act
  (exp) cadence and the Act engine never waits on phase boundaries.
- All PSUM [128,512] f32 tiles (qkv pq, rope pr, V-transpose, S, outproj
  po) share one 3-buffer pool (psS); avp accumulators keep 4 banks (psV);
  normalize-broadcast keeps 1 (psA).
- cos/sin loaded as TWO whole-tensor DMAs up-front (v1 chunk-loads built
  a 50us credit-semaphore chain that delayed the collectives); xt loads
  merged per (batch, c-block); staging is ONE DMA per q-chunk and
  att_load ONE DMA per (batch, head) via AP rearrange.
- V transposed on PE in bf16 (1 cyc/row), ones columns pre-memset into
  persistent v tiles so AV keeps the fused [65 x 512] denominator-row
  trick.
- PSUM->SBUF casts moved off the Act engine (exp is its critical work)
  onto DVE; big loads issue on the Act HW-DGE queue before exp starts,
  staging/att_load/out on the SP queue.
- Collectives issue on Pool, whose stream contains nothing else.
"""

import numpy as np
import ml_dtypes

import concourse.bass as bass
import concourse.mybir as mybir
import concourse.tile as tile
from concourse import bacc
from concourse.bass_utils import run_bass_kernel_spmd
from concourse.dve_ops import (RECIP_APPROX_FAST_CONSTS,
                               RECIPROCAL_APPROX_FAST)

F32 = mybir.dt.float32
BF16 = mybir.dt.bfloat16

B, T, C = 2, 2048, 1024
H, HD = 16, 64
NC = 8
HL = H // NC          # heads per core = 2
BT = B * T            # 4096
FQKV = 3 * HL * HD    # 384 rows of w_attn per core
TSH = BT // NC        # 512 output rows per core (256 per batch)
NCH = BT // 512       # 8 chunks of 512 t
ROPE_BASE = 10000.0
BF = ml_dtypes.bfloat16


def build():
    nc = bacc.Bacc(None, target_bir_lowering=False)

    xT_d = nc.dram_tensor("xT", [C, BT], BF16, kind="ExternalInput")
    wq_d = nc.dram_tensor("wqkvT", [C, FQKV], BF16, kind="ExternalInput")
    wp_d = nc.dram_tensor("wpT", [C, C], BF16, kind="ExternalInput")
    cos_d = nc.dram_tensor("cosT", [128, BT], BF16, kind="ExternalInput")
    sin_d = nc.dram_tensor("sinT", [128, BT], BF16, kind="ExternalInput")
    perm_d = nc.dram_tensor("permT", [128, 128], BF16, kind="ExternalInput")
    mask_d = nc.dram_tensor("masks", [4, 128, 512], BF16, kind="ExternalInput")
    id_d = nc.dram_tensor("identB", [128, 128], BF16, kind="ExternalInput")
    out_d = nc.dram_tensor("out", [TSH, C], BF16, kind="ExternalOutput")

    # block j of a2a_in = my 128 attention channels for core j's 256 t-rows
    # of batch b; block j of a2a_out = core j's channels for MY 256 t-rows.
    a2a_in = {(b_, h_): nc.dram_tensor(f"a2ain{b_}{h_}", [8, 64, 256], BF16)
              for b_ in range(2) for h_ in range(2)}
    a2a_out = {(b_, h_): nc.dram_tensor(f"a2aout{b_}{h_}", [8, 64, 256], BF16)
               for b_ in range(2) for h_ in range(2)}

    with tile.TileContext(nc) as tc:
        with (
            tc.tile_pool(name="persist", bufs=1) as pp,
            tc.tile_pool(name="work", bufs=4) as wk,
            tc.tile_pool(name="pts", bufs=20) as ptp,
            tc.tile_pool(name="psS", bufs=5, space="PSUM") as psS,
            tc.tile_pool(name="psV", bufs=2, space="PSUM") as psV,
            tc.tile_pool(name="psA", bufs=1, space="PSUM") as psA,
        ):
            # ---- constants / weights: few big DMAs, DRAM side rearranged
            # so all 8 c-blocks land in one SBUF tile per tensor ----
            wq_big = pp.tile([128, 8 * FQKV], BF16, name="wqb", tag="wqb")
            nc.sync.dma_start(
                wq_big[:, 0:4 * FQKV],
                wq_d[0:512].rearrange("(c p) f -> p c f", p=128))
            nc.scalar.dma_start(
                wq_big[:, 4 * FQKV:],
                wq_d[512:1024].rearrange("(c p) f -> p c f", p=128))
            id_sb = pp.tile([128, 128], BF16, name="id_sb", tag="id_sb")
            nc.sync.dma_start(id_sb[:], id_d[:])
            perm_sb = pp.tile([128, 128], BF16, name="perm_sb", tag="perm_sb")
            nc.sync.dma_start(perm_sb[:], perm_d[:])
            # xt: [128, 8*2048], one generation per batch; halves split
            # across the two HW-DGE queues so batch 0 lands in ~6us
            xt_big = [None]

            def xt_load(b):
                # 4 DMAs, 2 per HW queue, so the first qkv accumulation
                # chain can chase the c-blocks as they land
                t = pp.tile([128, 8 * 2048], BF16, name=f"xt{b}", tag="xt")
                for q_ in range(4):
                    eng = nc.sync if q_ % 2 == 0 else nc.scalar
                    eng.dma_start(
                        t[:, q_ * 2 * 2048:(q_ + 1) * 2 * 2048],
                        xT_d[q_ * 256:(q_ + 1) * 256,
                             b * 2048:(b + 1) * 2048].rearrange(
                            "(c p) t -> p c t", p=128))
                xt_big[0] = t

            xt_load(0)
            cos_sb = pp.tile([128, BT], BF16, name="cos_sb", tag="cos_sb")
            nc.scalar.dma_start(cos_sb[:], cos_d[:])
            sin_sb = pp.tile([128, BT], BF16, name="sin_sb", tag="sin_sb")
            nc.scalar.dma_start(sin_sb[:], sin_d[:])
            mask_big = pp.tile([128, 4 * 512], BF16, name="maskb", tag="maskb")
            nc.scalar.dma_start(
                mask_big[:], mask_d[:].rearrange("m p c -> p m c"))

            ones_f = pp.tile([128, 1], F32, name="ones_f", tag="ones_f")
            nc.vector.memset(ones_f[:], 1.0)
            ones_r = pp.tile([1, HD], mybir.dt.float32r, name="ones_r",
                             tag="ones_r")
            nc.vector.tensor_copy(ones_r[:],
                                  ones_f[0:1, 0:1].broadcast_to((1, HD)))

            # persistent V tiles [t,d]: cols 64 and 129 stay the memset 1.0
            # (denominator rows of the fused AV); memsets on the idle Pool
            v_sb = []
            for kb in range(32):
                v = pp.tile([128, 2 * (HD + 1)], BF16, name=f"v{kb}",
                            tag=f"v{kb}")
                nc.gpsimd.memset(v[:], 1.0)
                v_sb.append(v)

            qtc = [pp.tile([128, 512], BF16, name=f"qtc{i}", tag=f"qtc{i}")
                   for i in range(NCH)]
            ktc = [pp.tile([128, 512], BF16, name=f"ktc{i}", tag=f"ktc{i}")
                   for i in range(NCH)]
            vtc = [pp.tile([128, 512], BF16, name=f"vtc{i}", tag=f"vtc{i}")
                   for i in range(NCH)]
            fdst = [qtc, ktc, vtc]

            att_sb = {}
            for b in range(2):
                att_sb[b] = pp.tile([128, 2048], BF16, name=f"att{b}",
                                    tag=f"att{b}")
            wp_big = [None]

            def wp_load():
                t = pp.tile([128, 8 * C], BF16, name="wpb", tag="wpb")
                nc.scalar.dma_start(
                    t[:], wp_d[:].rearrange("(c p) o -> p c o", p=128))
                wp_big[0] = t

            # ---- building blocks ----
            def qkv_mms(pq, b, f, tq, cs):
                xt = xt_big[0]
                for c in cs:
                    nc.tensor.matmul(
                        pq[:],
                        wq_big[:, c * FQKV + f * 128:c * FQKV + f * 128 + 128],
                        xt[:, c * 2048 + tq * 512:c * 2048 + tq * 512 + 512],
                        start=(c == 0), stop=(c == 7))

            def qkv_piece(b, f, tq):
                """one [128, 512] slice of the qkv projection for batch b;
                lead-in (b=0) casts go to the then-idle Act engine"""
                pq = psS.tile([128, 512], F32, name=f"pq{b}{f}{tq}",
                              tag="ps_s")
                qkv_mms(pq, b, f, tq, range(8))
                if b == 0:
                    nc.scalar.copy(fdst[f][b * 4 + tq][:], pq[:])
                else:
                    nc.vector.tensor_copy(fdst[f][b * 4 + tq][:], pq[:])

            def qkv_quanta(b, f, tq):
                """same piece as 4 filler quanta of ~2 matmuls (~430ns),
                sized to hide in the exp-latency bubble of one kb block"""
                hold = {}

                def q_first():
                    hold['pq'] = psS.tile([128, 512], F32,
                                          name=f"pq{b}{f}{tq}", tag="ps_s")
                    qkv_mms(hold['pq'], b, f, tq, (0, 1))

                def q_mid(cs):
                    def g():
                        qkv_mms(hold['pq'], b, f, tq, cs)
                    return g

                def q_last():
                    qkv_mms(hold['pq'], b, f, tq, (6, 7))
                    nc.vector.tensor_copy(fdst[f][b * 4 + tq][:],
                                          hold['pq'][:])
                return [q_first, q_mid((2, 3)), q_mid((4, 5)), q_last]

            def rope_mm(ch, which):
                """RoPE one chunk of q or k, in place"""
                tcl = qtc if which == "q" else ktc
                src = tcl[ch]
                pr = psS.tile([128, 512], F32, name=f"pr{which}{ch}",
                              tag="ps_s")
                nc.tensor.matmul(pr[:], perm_sb[:], src[:],
                                 start=True, stop=True)
                rot = wk.tile([128, 512], BF16, name=f"rot{which}{ch}",
                              tag="rot")
                nc.vector.tensor_mul(rot[:], pr[:],
                                     sin_sb[:, ch * 512:(ch + 1) * 512])
                nc.vector.tensor_mul(src[:], src[:],
                                     cos_sb[:, ch * 512:(ch + 1) * 512])
                nc.vector.tensor_add(src[:], src[:], rot[:])

            def vt_block(kb):
                """V block kb -> [t, d] via bf16 PE transpose"""
                pvt = psS.tile([128, 512], BF16, name=f"pvt{kb}", tag="ps_s")
                nc.tensor.transpose(
                    pvt[:, 0:128],
                    vtc[kb // 4][:, (kb % 4) * 128:(kb % 4 + 1) * 128],
                    id_sb[:])
                eng = nc.scalar if kb < 16 else nc.vector
                if eng is nc.scalar:
                    nc.scalar.copy(v_sb[kb][:, 0:HD], pvt[:, 0:HD])
                    nc.scalar.copy(v_sb[kb][:, HD + 1:2 * HD + 1],
                                   pvt[:, HD:2 * HD])
                else:
                    nc.vector.tensor_copy(v_sb[kb][:, 0:HD], pvt[:, 0:HD])
                    nc.vector.tensor_copy(v_sb[kb][:, HD + 1:2 * HD + 1],
                                          pvt[:, HD:2 * HD])

            def normalize(b, h, qc, avq):
                den = wk.tile([1, 512], F32, name=f"den{b}{h}{qc}", tag="den")
                nc.vector.tensor_copy(den[:], avq[HD:HD + 1, :])
                avs = wk.tile([HD, 512], F32, name=f"avs{b}{h}{qc}",
                              tag="avs")
                nc.vector.tensor_copy(avs[:], avq[0:HD, :])
                recr = wk.tile([1, 512], mybir.dt.float32r,
                               name=f"recr{b}{h}{qc}", tag="recr")
                cst = RECIP_APPROX_FAST_CONSTS
                nc.vector._custom_dve(RECIPROCAL_APPROX_FAST, out=recr[:],
                                      in0=den[:], s0=cst["s0"], s1=cst["s1"],
                                      imm2=cst["imm2"])
                bc = psA.tile([HD, 512], F32, name=f"bc{b}{h}{qc}",
                              tag="ps_a")
                nc.tensor.matmul(bc[:], ones_r[:], recr[:],
                                 start=True, stop=True)
                attn = wk.tile([HD, 512], BF16, name=f"attn{b}{h}{qc}",
                               tag="attn")
                nc.vector.tensor_mul(attn[:], avs[0:HD, :], bc[:])
                # one DMA: [64, (2,256)] -> rows 128qc..128qc+128 of a2a_in
                nc.sync.dma_start(
                    a2a_in[b, h][2 * qc:2 * qc + 2].rearrange(
                        "h p c -> p h c"),
                    attn[:].rearrange("p (h c) -> p h c", h=2))

            def a2a_issue(b, h):
                nc.gpsimd.collective_compute(
                    "AllToAll",
                    mybir.AluOpType.bypass,
                    replica_groups=[list(range(NC))],
                    ins=[a2a_in[b, h][:]],
                    outs=[a2a_out[b, h][:]],
                )

            def att_load(b, h):
                # Pool engine: its stream holds only collectives/memsets, so
                # waiting on the collective blocks nothing else
                nc.gpsimd.dma_start(
                    att_sb[b][HD * h:HD * (h + 1), :].rearrange(
                        "p (c k) -> p c k", c=8),
                    a2a_out[b, h][:].rearrange("c p k -> p c k"))

            dummy_n = [0]

            def dummy_mm():
                """clock-warmer: one real-shaped matmul into a never-read
                psS tile; fills the ~300ns exp-latency bubble of a block"""
                dummy_n[0] += 1
                d = psS.tile([128, 512], F32, name=f"dm{dummy_n[0]}",
                             tag="ps_s")
                nc.tensor.matmul(d[:], wq_big[:, 0:128],
                                 xt_big[0][:, 0:512], start=True, stop=True)

            def op_mms(po, b, tb, j, cs):
                for c in cs:
                    nc.tensor.matmul(
                        po[:],
                        att_sb[b][:, 256 * c + 128 * tb:
                                  256 * c + 128 * tb + 128],
                        wp_big[0][:, c * C + j * 512:c * C + j * 512 + 512],
                        start=(c == 0), stop=(c == 7))

            def op_finish(po, b, tb, j):
                # keep the Act engine free for exp: casts on DVE, DMA
                # issues on the SP queue
                ot = wk.tile([128, 512], BF16, name=f"ot{b}{tb}{j}",
                             tag="ot")
                nc.vector.tensor_copy(ot[:], po[:])
                nc.sync.dma_start(
                    out_d[b * 256 + tb * 128:b * 256 + (tb + 1) * 128,
                          j * 512:(j + 1) * 512], ot[:])

            def outproj_piece(b, tb, j):
                po = psS.tile([128, 512], F32, name=f"po{b}{tb}{j}",
                              tag="ps_s")
                op_mms(po, b, tb, j, range(8))
                op_finish(po, b, tb, j)

            def outproj_quanta(b, tb, j):
                hold = {}

                def q_first():
                    hold['po'] = psS.tile([128, 512], F32,
                                          name=f"po{b}{tb}{j}", tag="ps_s")
                    op_mms(hold['po'], b, tb, j, (0, 1))

                def q_mid(cs):
                    def g():
                        op_mms(hold['po'], b, tb, j, cs)
                    return g

                def q_last():
                    op_mms(hold['po'], b, tb, j, (6, 7))
                    op_finish(hold['po'], b, tb, j)
                return [q_first, q_mid((2, 3)), q_mid((4, 5)), q_last]

            # ---- attention, qc-outer: one avp accumulator live at a time
            # (psV=2 banks) buys a 5-deep S pipeline (psS=5) so the PE can
            # run ahead of the exp cadence; fillers injected every 2nd kb
            def attention(b, fillers):
                for h in range(HL):
                    hp = h * 64

                    def s_block(kb, qc, b=b, h=h, hp=hp):
                        kch = ktc[b * 4 + kb // 4]
                        koff = (kb % 4) * 128
                        m = kb % 4 if qc == kb // 4 else 0
                        c0 = 128 * m
                        sps = psS.tile([128, 512], F32,
                                       name=f"s{b}{h}{kb}{qc}", tag="ps_s")
                        nc.tensor.matmul(
                            sps[:, c0:512],
                            kch[hp:hp + 64, koff:koff + 128],
                            qtc[b * 4 + qc][hp:hp + 64, c0:512],
                            start=True, stop=True,
                        )
                        pt = ptp.tile([128, 512], BF16,
                                      name=f"pt{b}{h}{kb}{qc}", tag="pt")
                        nc.scalar.activation(
                            pt[:, c0:512], sps[:, c0:512],
                            mybir.ActivationFunctionType.Exp,
                            scale=0.125,
                        )
                        if qc == kb // 4:
                            nc.vector.tensor_mul(
                                pt[:, c0:512], pt[:, c0:512],
                                mask_big[:, (kb % 4) * 512 + c0:
                                         (kb % 4) * 512 + 512])
                        return pt, c0

                    for qc in range(4):
                        avq = psV.tile([HD + 1, 512], F32,
                                       name=f"av{b}{h}{qc}", tag="ps_av")
                        nkb = 4 * qc + 4

                        def av_block(kb, pt, c0, avq=avq, b=b, h=h, qc=qc):
                            nc.tensor.matmul(
                                avq[:, c0:512],
                                v_sb[b * 16 + kb][:, h * (HD + 1):
                                                  (h + 1) * (HD + 1)],
                                pt[:, c0:512],
                                start=(kb == 0), stop=(kb == 4 * qc + 3),
                                skip_group_check=bool(c0),
                            )

                        prev = s_block(0, qc)
                        for kb in range(1, nkb):
                            cur = s_block(kb, qc)
                            av_block(kb - 1, *prev)
                            if fillers:
                                fillers.pop(0)()
                            prev = cur
                        av_block(nkb - 1, *prev)
                        normalize(b, h, qc, avq)
                        if fillers:
                            fillers.pop(0)()
                    a2a_issue(b, h)

            # ================= main flow =================
            # batch-0 qkv (q, k), rope b0 threaded through the v slices
            for f in (0, 1):
                for tq in range(4):
                    qkv_piece(0, f, tq)
            rope_mm(0, "q")
            qkv_piece(0, 2, 0)
            rope_mm(0, "k")
            qkv_piece(0, 2, 1)
            rope_mm(1, "q")
            qkv_piece(0, 2, 2)
            rope_mm(1, "k")
            qkv_piece(0, 2, 3)
            rope_mm(2, "q")
            for kb in range(0, 4):
                vt_block(kb)
            rope_mm(2, "k")
            for kb in range(4, 8):
                vt_block(kb)
            rope_mm(3, "q")
            for kb in range(8, 12):
                vt_block(kb)
            rope_mm(3, "k")
            for kb in range(12, 16):
                vt_block(kb)

            # batch-1 x loads go out now, wp behind them
            xt_load(1)
            wp_load()

            # fillers for attention(0): batch-1 qkv + rope + V transposes,
            # as ~430ns quanta sized to the per-block exp bubble
            f0 = []
            for tq in range(4):
                f0 += qkv_quanta(1, 0, tq)
            f0 += qkv_quanta(1, 1, 0)
            f0.append(lambda: rope_mm(4, "q"))
            f0 += qkv_quanta(1, 1, 1)
            f0.append(lambda: rope_mm(5, "q"))
            f0 += qkv_quanta(1, 1, 2)
            f0.append(lambda: rope_mm(6, "q"))
            f0 += qkv_quanta(1, 1, 3)
            f0.append(lambda: rope_mm(7, "q"))
            f0.append(lambda: rope_mm(4, "k"))
            f0 += qkv_quanta(1, 2, 0)
            f0.append(lambda: rope_mm(5, "k"))
            f0.append(lambda: vt_block(16))
            f0.append(lambda: vt_block(17))
            f0 += qkv_quanta(1, 2, 1)
            f0.append(lambda: rope_mm(6, "k"))
            f0.append(lambda: vt_block(18))
            f0.append(lambda: vt_block(19))
            f0 += qkv_quanta(1, 2, 2)
            f0.append(lambda: rope_mm(7, "k"))
            f0.append(lambda: vt_block(20))
            f0.append(lambda: vt_block(21))
            f0 += qkv_quanta(1, 2, 3)
            for kb in range(22, 32):
                f0.append(lambda kb=kb: vt_block(kb))

            attention(0, f0)
            while f0:
                f0.pop(0)()

            att_load(0, 0)
            att_load(0, 1)

            # fillers for attention(1): 1-matmul dummies bridge the exp
            # bubbles; batch-0 outproj quanta go late in head 1 so
            # a2a(0,1) has certainly landed (pops 41-80 are head 1)
            # dummies every OTHER block: every-slot dummies make the PE
            # stream (~1030ns/block incl. Ldweights) overtake the 686ns
            # exp cadence; bare blocks let the clock decay. Alternate.
            # NO collective-dependent filler in the stream: a peer-skewed
            # a2a(0,1) must not stall the attention pipeline (that stall
            # cascades into a late a2a(1,1) trigger and ~+30us runs).
            noop = lambda: None
            f1 = []
            for _ in range(40):
                f1 += [dummy_mm, noop]

            attention(1, f1)
            while f1:
                f1.pop(0)()

            att_load(1, 0)
            att_load(1, 1)
            # batch-0 outproj rides inside the a2a(1,1) latency window
            for tb in range(2):
                for j in range(2):
                    outproj_piece(0, tb, j)
            for tb in range(2):
                for j in range(2):
                    outproj_piece(1, tb, j)

    nc.finalize()
    return nc


def host_inputs(x, w_attn, w_proj):
    x2 = np.ascontiguousarray(x.reshape(BT, C).T).astype(BF)   # [C, BT]

    inv = 1.0 / (ROPE_BASE ** (np.arange(0, HD, 2, dtype=np.float32) / HD))
    tpos = np.arange(T, dtype=np.float32)
    freqs = tpos[:, None] * inv[None, :]                  # [T, 32]
    emb = np.concatenate([freqs, freqs], axis=-1)         # [T, 64]
    cosT = np.cos(emb).T.astype(np.float32)               # [64, T]
    sinT = np.sin(emb).T.astype(np.float32)
    cos_full = np.ascontiguousarray(np.tile(cosT, (2, B))).astype(BF)
    sin_full = np.ascontiguousarray(np.tile(sinT, (2, B))).astype(BF)

    m64 = np.zeros((HD, HD), dtype=np.float32)
    half = HD // 2
    for d in range(half):
        m64[d, d + half] = -1.0
        m64[d + half, d] = 1.0
    perm = np.zeros((128, 128), dtype=np.float32)
    perm[0:HD, 0:HD] = m64
    perm[HD:128, HD:128] = m64
    permT = np.ascontiguousarray(perm.T).astype(BF)

    masks = np.zeros((4, 128, 512), dtype=np.float32)
    qi = np.arange(512)[None, :]
    ki = np.arange(128)[:, None]
    for m in range(4):
        masks[m] = (qi - ki >= m * 128).astype(np.float32)
    masks = masks.astype(BF)

    identB = np.eye(128, dtype=np.float32).astype(BF)
    wpT = np.ascontiguousarray(w_proj.T).astype(BF)       # [c, o]

    in_maps = []
    for i in range(NC):
        r0 = i * (HL * HD)
        wq = w_attn[r0:r0 + HL * HD, :]
        wk_ = w_attn[C + r0:C + r0 + HL * HD, :]
        wv = w_attn[2 * C + r0:2 * C + r0 + HL * HD, :]
        wqkvT = np.ascontiguousarray(
            np.concatenate([wq, wk_, wv], axis=0).T).astype(BF)
        in_maps.append({
            "xT": x2, "wqkvT": wqkvT, "wpT": wpT,
            "cosT": cos_full, "sinT": sin_full, "permT": permT,
            "masks": masks, "identB": identB,
        })
    return in_maps


_NC_CACHE = None


def _get_nc():
    global _NC_CACHE
    if _NC_CACHE is None:
        _NC_CACHE = build()
    return _NC_CACHE


def run(x, w_attn, w_proj, trace=False):
    nc = _get_nc()
    in_maps = host_inputs(np.asarray(x), np.asarray(w_attn),
                          np.asarray(w_proj))
    res = run_bass_kernel_spmd(nc, in_maps, list(range(NC)), trace=trace)
    out = np.empty((B, T, C), dtype=np.float32)
    piece = T // NC
    for i in range(NC):
        sh = np.asarray(res.results[i]["out"]).astype(np.float32)
        out[0, i * piece:(i + 1) * piece] = sh[0:piece]
        out[1, i * piece:(i + 1) * piece] = sh[piece:2 * piece]
    return out, res


def kernel(x, w_attn, w_proj):
    out, _ = run(x, w_attn, w_proj, trace=False)
    return out
